# revision 1
# baseline (speedup 1.0000x reference)
"""Relative-position attention (Music-Transformer style skew) + LayerNorm,
distributed over 8 TRN2 NeuronCores.

Sharding: data-parallel over batch (B=4) x tensor-parallel over head-halves
(H=8 -> 2 groups of 4). Core c handles batch b=c//2, heads [4*(c%2), 4*(c%2)+4),
producing output channels [256*(c%2), +256) of y[b]. The final LayerNorm needs
full-E stats, exchanged via a tiny pairwise AllReduce of (sum, sumsq).

Skew trick: Srel[i,j] = F[(i+1)*S + j] where F is the row-major flat view of
the padded matrix P[i, 0]=0, P[i, 1+l]=QEr[i, l] (P is [S, S+1]). We bounce P
through DRAM in bf16; the skewed read back is a plain strided DMA.

All transposes are plain matmuls (lhsT.T @ I) rather than transpose-mode
matmuls: walrus gives the LDWEIGHTS lowering very few sync-wait slots, and
"absorber" PE nops (which read recently-written tiles) keep every matmul at
<=1 wait.
"""

import numpy as np

import concourse.bass as bass
import concourse.mybir as mybir
from concourse import masks
from concourse.tile import TileContext

F32 = mybir.dt.float32
F32R = mybir.dt.float32r
BF16 = mybir.dt.bfloat16

B, S, E, H = 4, 2048, 512, 8
HD = E // H          # 64
HLOC = 4             # heads per core
CH = HLOC * HD       # 256 output channels per core
SCALE = float(E) ** -0.5
EPS = 1e-5
N_CORES = 8


def build_nc(s=S, n_cores=N_CORES, debug=False, legalize=True):
    """Build the per-core Bass graph (SPMD: same graph on all cores)."""
    nc = bass.Bass(target_bir_lowering=False, debug=debug)

    SB = s // 128        # number of 128-row blocks
    KC = s // 512        # number of 512-col chunks

    x_d = nc.declare_dram_parameter("x", [s, E], F32, isOutput=False)
    wq_d = nc.declare_dram_parameter("wq", [CH, E], F32, isOutput=False)
    wk_d = nc.declare_dram_parameter("wk", [CH, E], F32, isOutput=False)
    wv_d = nc.declare_dram_parameter("wv", [CH, E], F32, isOutput=False)
    er_d = nc.declare_dram_parameter("er", [s, HD], F32, isOutput=False)
    gamma_d = nc.declare_dram_parameter("gamma", [1, CH], F32, isOutput=False)
    beta_d = nc.declare_dram_parameter("beta", [1, CH], F32, isOutput=False)
    out_d = nc.declare_dram_parameter("out", [s, CH], F32, isOutput=True)

    # Padded-QEr bounce buffers, one per head, flat [S*(S+1)] bf16.
    p_d = nc.dram_tensor("pbuf", [HLOC, s * (s + 1)], BF16)
    cc_in = nc.dram_tensor("cc_in", [s, 2], F32)
    cc_out = nc.dram_tensor("cc_out", [s, 2], F32)

    pairs = [[2 * i, 2 * i + 1] for i in range(n_cores // 2)]

    with TileContext(nc) as tc:
        with (
            tc.tile_pool(name="const", bufs=1) as const_pool,
            tc.tile_pool(name="persist", bufs=1) as pp,
        ):
            ident_f32 = const_pool.tile([128, 128], F32)
            ident_bf16 = const_pool.tile([128, 128], BF16)
            masks.make_identity(nc, ident_f32[:])
            masks.make_identity(nc, ident_bf16[:])
            gamma_bc = const_pool.tile([128, CH], F32)
            beta_bc = const_pool.tile([128, CH], F32)
            eps_t = const_pool.tile([128, 1], F32)
            nc.gpsimd.memset(eps_t[:], EPS)
            nc.sync.dma_start(gamma_bc[:], gamma_d[:].broadcast_to((128, CH)))
            nc.sync.dma_start(beta_bc[:], beta_d[:].broadcast_to((128, CH)))

            # ---- persistent SBUF tensors ----
            xT = [pp.tile([128, s], F32R, tag=f"xT{ec}", name=f"xT{ec}")
                  for ec in range(4)]
            wT = {
                w: [pp.tile([128, CH], F32R, tag=f"{w}T{ec}", name=f"{w}T{ec}")
                    for ec in range(4)]
                for w in ("wq", "wk", "wv")
            }
            # ErT replicated into both partition halves so every head's
            # matmul finds it at its own base partition (PE requires
            # lhsT/rhs base partitions to match).
            erT = pp.tile([128, s], F32R, tag="erT")
            qT = [pp.tile([128, s], F32R, tag=f"qT{oc}", name=f"qT{oc}")
                  for oc in range(2)]
            kT = [pp.tile([128, s], F32R, tag=f"kT{oc}", name=f"kT{oc}")
                  for oc in range(2)]
            # v with a ones column appended per head: [128, HLOC*(HD+1)] bf16
            vaug = [pp.tile([128, HLOC * (HD + 1)], BF16, tag=f"va{sb}",
                            name=f"va{sb}") for sb in range(SB)]
            outp = [pp.tile([128, CH], F32, tag=f"op{sb}", name=f"op{sb}")
                    for sb in range(SB)]

            # ================= setup: load + transpose =================
            with (
                tc.tile_pool(name="ld", bufs=4) as ld_pool,
                tc.tile_pool(name="ps_set", bufs=4, space="PSUM") as ps_set,
            ):
                # Warm-up: absorb the Pool (identity-creation) dependency
                # into PE's observed clock.
                warm = ps_set.tile([128, 128], F32, tag="pset")
                nc.tensor.matmul(
                    warm[:], ident_f32[:], ident_f32[:], start=True, stop=True)

                # xT[ec][:, i*128:(i+1)*128] = x[i-block, ec-block].T
                for sb in range(SB):
                    xt = ld_pool.tile([128, E], F32, tag="xld")
                    nc.sync.dma_start(xt[:], x_d[sb * 128:(sb + 1) * 128, :])
                    for ec in range(4):
                        pst = ps_set.tile([128, 128], F32, tag="pset")
                        nc.tensor.transpose(
                            pst[:], xt[:, ec * 128:(ec + 1) * 128],
                            ident_f32[:])
                        nc.vector.tensor_copy(
                            xT[ec][:, sb * 128:(sb + 1) * 128], pst[:])
                # weights
                for w_name, w_d in (("wq", wq_d), ("wk", wk_d), ("wv", wv_d)):
                    for pc in range(CH // 128):
                        wt = ld_pool.tile([128, E], F32, tag="wld")
                        nc.sync.dma_start(
                            wt[:], w_d[pc * 128:(pc + 1) * 128, :])
                        for ec in range(4):
                            pst = ps_set.tile([128, 128], F32, tag="pset")
                            nc.tensor.transpose(
                                pst[:], wt[:, ec * 128:(ec + 1) * 128],
                                ident_f32[:])
                            nc.vector.tensor_copy(
                                wT[w_name][ec][:, pc * 128:(pc + 1) * 128],
                                pst[:])
                # Er: transpose into both partition halves, one DVE copy
                for sb in range(SB):
                    et = ld_pool.tile([128, HD], F32, tag="eld")
                    nc.sync.dma_start(et[:], er_d[sb * 128:(sb + 1) * 128, :])
                    pst = ps_set.tile([128, 128], F32, tag="psete")
                    nc.tensor.transpose(pst[0:64, :], et[:], ident_f32[:])
                    nc.tensor.matmul(
                        pst[64:128, :], et[:], ident_f32[:],
                        start=True, stop=True)
                    nc.vector.tensor_copy(
                        erT[:, sb * 128:(sb + 1) * 128], pst[:])

            # ================= projections + attention =================
            with (
                tc.tile_pool(name="wrk", bufs=4) as wrk,
                tc.tile_pool(name="wrk2", bufs=4) as wrk2,
                tc.tile_pool(name="wrk3", bufs=4) as wrk3,
                tc.tile_pool(name="small", bufs=8) as small,
            ):
                with tc.tile_pool(
                        name="ps_pj", bufs=4, space="PSUM") as ps_pj:
                    # qT / kT: [oc*128+p, t] = sum_e W[oc*128+p, e] x[t, e]
                    for dst, w_name in ((qT, "wq"), (kT, "wk")):
                        for oc in range(2):
                            for sc in range(KC):
                                ps = ps_pj.tile([128, 512], F32, tag="pj")
                                for ec in range(4):
                                    nc.tensor.matmul(
                                        ps[:],
                                        wT[w_name][ec][:, oc * 128:
                                                       (oc + 1) * 128],
                                        xT[ec][:, sc * 512:(sc + 1) * 512],
                                        start=(ec == 0), stop=(ec == 3))
                                nc.scalar.copy(
                                    dst[oc][:, sc * 512:(sc + 1) * 512],
                                    ps[:])
                    # v natural + ones column, bf16
                    for sb in range(SB):
                        ps = ps_pj.tile([128, CH], F32, tag="pj")
                        for ec in range(4):
                            nc.tensor.matmul(
                                ps[:],
                                xT[ec][:, sb * 128:(sb + 1) * 128],
                                wT["wv"][ec][:],
                                start=(ec == 0), stop=(ec == 3))
                        for h in range(HLOC):
                            nc.scalar.copy(
                                vaug[sb][:, h * (HD + 1):h * (HD + 1) + HD],
                                ps[:, h * HD:(h + 1) * HD])
                            nc.vector.memset(
                                vaug[sb][:, h * (HD + 1) + HD:
                                         (h + 1) * (HD + 1)],
                                1.0)

                # ---------------- per-head attention ----------------
                from contextlib import ExitStack
                att_stk = ExitStack()
                ps_qk = att_stk.enter_context(tc.tile_pool(
                    name="ps_qk", bufs=3, space="PSUM"))
                ps_qa = att_stk.enter_context(tc.tile_pool(
                    name="ps_qa", bufs=2, space="PSUM"))
                ps_tr = att_stk.enter_context(tc.tile_pool(
                    name="ps_tr", bufs=2, space="PSUM"))
                ps_av = att_stk.enter_context(tc.tile_pool(
                    name="ps_av", bufs=1, space="PSUM"))
                HW2 = s // 2
                CK = min(512, HW2)
                NTH = HW2 // 128

                def head_slices(h):
                    oc, po = h // 2, (h % 2) * 64
                    return (qT[oc][po:po + 64, :], kT[oc][po:po + 64, :], po)

                def phase_a_block(h, sb):
                    """exp-input QEr block -> padded P[h] (bf16)."""
                    qTh, _, po = head_slices(h)
                    pexp = wrk.tile([128, s + 1], BF16, tag="pexp",
                                    name="pexp")
                    nc.vector.memset(pexp[:, 0:1], 0.0)
                    for qt in range(s // CK):
                        ps = ps_qa.tile([128, CK], F32, tag="qa", name="psA")
                        c0 = qt * CK
                        nc.tensor.matmul(
                            ps[:],
                            qTh[:, sb * 128:(sb + 1) * 128],
                            erT[po:po + 64, c0:c0 + CK],
                            start=True, stop=True)
                        # split the PSUM->bf16 cast between ACT and DVE
                        if qt % 2 == 0:
                            nc.scalar.copy(pexp[:, 1 + c0:1 + c0 + CK], ps[:])
                        else:
                            nc.vector.tensor_copy(
                                pexp[:, 1 + c0:1 + c0 + CK], ps[:])
                    nc.sync.dma_start(
                        p_d[h, sb * 128 * (s + 1):(sb * 128 + 128) * (s + 1)]
                        .rearrange("(r c) -> r c", c=s + 1),
                        pexp[:])

                def phase_b_block(h, sb):
                    """scores -> softmax -> AV for one q-block.

                    Transposes and AV matmuls are emitted as one contiguous
                    PE burst (>3.4us) so the HAM clock-gate upshifts."""
                    qTh, kTh, _ = head_slices(h)
                    pc_av = ps_av.tile([128, HD + 1], F32, tag="av",
                                       name="pc_av")
                    base = (sb * 128 + 1) * s
                    srel = wrk2.tile([128, s], BF16, tag="srel", name="srel")
                    nc.sync.dma_start(
                        srel[:],
                        p_d[h, base:base + 128 * s]
                        .rearrange("(r c) -> r c", c=s))
                    CKB = min(512, HW2)
                    sc_ts = []
                    for hf in range(2):
                        sc_t = wrk2.tile([128, HW2], BF16, tag=f"sc{hf}",
                                         name="sc_t")
                        sc_ts.append(sc_t)
                        for kc in range(HW2 // CKB):
                            c0 = hf * HW2 + kc * CKB
                            ps = ps_qk.tile([128, CKB], F32, tag="qk",
                                            name="psB")
                            nc.tensor.matmul(
                                ps[:],
                                qTh[:, sb * 128:(sb + 1) * 128],
                                kTh[:, c0:c0 + CKB],
                                start=True, stop=True)
                            nc.vector.tensor_add(
                                sc_t[:, kc * CKB:(kc + 1) * CKB], ps[:],
                                srel[:, c0:c0 + CKB])
                    psts = []
                    tpss = []
                    for hf in range(2):
                        pst = ps_tr.tile([128, HW2], BF16, tag="tr",
                                         name="pst")
                        psts.append(pst)
                        for t in range(NTH):
                            nc.tensor.transpose(
                                pst[:, t * 128:(t + 1) * 128],
                                sc_ts[hf][:, t * 128:(t + 1) * 128],
                                ident_bf16[:])
                        tps = wrk3.tile([128, HW2], BF16, tag=f"tps{hf}",
                                        name="tps")
                        tpss.append(tps)
                        nc.scalar.activation(
                            tps[:], pst[:],
                            mybir.ActivationFunctionType.Exp, scale=SCALE)
                    for hf in range(2):
                        for t in range(NTH):
                            ci = hf * NTH + t
                            nc.tensor.matmul(
                                pc_av[:],
                                tpss[hf][:, t * 128:(t + 1) * 128],
                                vaug[ci][:, h * (HD + 1):(h + 1) * (HD + 1)],
                                start=(ci == 0), stop=(ci == 2 * NTH - 1))
                    rinv = small.tile([128, 1], F32, tag="rinv", name="rinv")
                    nc.vector.reciprocal(rinv[:], pc_av[:, HD:HD + 1])
                    nc.vector.tensor_scalar_mul(
                        outp[sb][:, h * HD:(h + 1) * HD],
                        pc_av[:, 0:HD], rinv[:])

                # software pipeline: phase A of head h+1 interleaves with
                # phase B of head h, keeping PE dense (HAM clock warm)
                def ln_stats_block(sb):
                    s1 = small.tile([128, 1], F32, tag="s1", name="s1")
                    nc.vector.reduce_sum(
                        s1[:], outp[sb][:], axis=mybir.AxisListType.X)
                    sq = small.tile([128, 1], F32, tag="sq", name="sq")
                    scr = wrk.tile([128, CH], F32, tag="scr", name="scr")
                    nc.scalar.activation(
                        scr[:], outp[sb][:],
                        mybir.ActivationFunctionType.Square, accum_out=sq[:])
                    nc.sync.dma_start(
                        cc_in[sb * 128:(sb + 1) * 128, 0:1], s1[:])
                    nc.sync.dma_start(
                        cc_in[sb * 128:(sb + 1) * 128, 1:2], sq[:])

                for sb in range(SB):
                    phase_a_block(0, sb)
                for h in range(HLOC):
                    for sb in range(SB):
                        phase_b_block(h, sb)
                        if h + 1 < HLOC:
                            phase_a_block(h + 1, sb)
                        else:
                            ln_stats_block(sb)

                att_stk.close()
                # ================= LayerNorm =================
                nc.gpsimd.collective_compute(
                    "AllReduce", mybir.AluOpType.add,
                    replica_groups=pairs,
                    ins=[cc_in[:].opt()], outs=[cc_out[:].opt()])
                for sb in range(SB):
                    st = small.tile([128, 2], F32, tag="st")
                    nc.sync.dma_start(st[:], cc_out[sb * 128:(sb + 1) * 128, :])
                    mean = small.tile([128, 1], F32, tag="mean")
                    nc.vector.tensor_scalar_mul(mean[:], st[:, 0:1], 1.0 / E)
                    ex2 = small.tile([128, 1], F32, tag="ex2")
                    nc.vector.tensor_scalar_mul(ex2[:], st[:, 1:2], 1.0 / E)
                    msq = small.tile([128, 1], F32, tag="msq")
                    nc.vector.tensor_mul(msq[:], mean[:], mean[:])
                    var = small.tile([128, 1], F32, tag="var")
                    nc.vector.tensor_sub(var[:], ex2[:], msq[:])
                    std = small.tile([128, 1], F32, tag="std")
                    nc.scalar.activation(
                        std[:], var[:],
                        mybir.ActivationFunctionType.Sqrt, bias=eps_t[:])
                    rstd = small.tile([128, 1], F32, tag="rstd")
                    nc.vector.reciprocal(rstd[:], std[:])
                    tmp = wrk.tile([128, CH], F32, tag="tmp")
                    nc.vector.tensor_scalar(
                        tmp[:], outp[sb][:], mean[:], rstd[:],
                        op0=mybir.AluOpType.subtract,
                        op1=mybir.AluOpType.mult)
                    y1 = wrk2.tile([128, CH], F32, tag="y1")
                    nc.vector.tensor_mul(y1[:], tmp[:], gamma_bc[:])
                    y2 = wrk3.tile([128, CH], F32, tag="y2")
                    nc.vector.tensor_add(y2[:], y1[:], beta_bc[:])
                    nc.sync.dma_start(out_d[sb * 128:(sb + 1) * 128, :], y2[:])

    if legalize:
        _legalize_waits(nc)
    return nc


def _legalize_waits(nc):
    """walrus's codegen accepts at most one sync wait on most instruction
    structs; hoist extra waits onto NoOps inserted just before, on the
    same engine queue (program order preserves the semantics)."""
    n = 0
    keep = set()
    for bb in nc.main_func.blocks:
        out = []
        for inst in bb.instructions:
            si = inst.sync_info
            if (inst.opcode not in keep and si is not None
                    and si.on_wait and len(si.on_wait) > 1):
                for w in si.on_wait[:-1]:
                    nop = mybir.InstNoOp(
                        name=f"I-mmw{n}", ins=[], outs=[])
                    n += 1
                    nop.engine = inst.engine
                    nop.sync_info = mybir.SyncInfo(
                        on_wait=[w], on_update=[])
                    out.append(nop)
                si.on_wait = [si.on_wait[-1]]
            out.append(inst)
        bb.instructions = out


_NC_CACHE = {}


def _get_nc(s=S, n_cores=N_CORES):
    key = (s, n_cores)
    if key not in _NC_CACHE:
        _NC_CACHE[key] = build_nc(s, n_cores)
    return _NC_CACHE[key]


def make_in_maps(x, Wq, Wk, Wv, Er, gamma, beta, n_cores=N_CORES):
    in_maps = []
    for c in range(n_cores):
        b, hg = c // 2, c % 2
        sl = slice(hg * CH, (hg + 1) * CH)
        in_maps.append({
            "x": np.ascontiguousarray(x[b], dtype=np.float32),
            "wq": np.ascontiguousarray(Wq[sl], dtype=np.float32),
            "wk": np.ascontiguousarray(Wk[sl], dtype=np.float32),
            "wv": np.ascontiguousarray(Wv[sl], dtype=np.float32),
            "er": np.ascontiguousarray(Er, dtype=np.float32),
            "gamma": np.ascontiguousarray(gamma[sl], dtype=np.float32)[None, :],
            "beta": np.ascontiguousarray(beta[sl], dtype=np.float32)[None, :],
        })
    return in_maps


def assemble(results, n_cores=N_CORES, s=S):
    y = np.empty((n_cores // 2, s, E), np.float32)
    for c in range(n_cores):
        y[c // 2, :, (c % 2) * CH:(c % 2 + 1) * CH] = results[c]["out"]
    return y


def kernel(**inputs):
    from concourse.bass_utils import run_bass_kernel_spmd
    nc = _get_nc()
    in_maps = make_in_maps(
        inputs["x"], inputs["Wq"], inputs["Wk"], inputs["Wv"],
        inputs["Er"], inputs["gamma"], inputs["beta"])
    res = run_bass_kernel_spmd(nc, in_maps, list(range(N_CORES)))
    return assemble(res.results)

